# revision 29
# baseline (speedup 1.0000x reference)
"""Causal self-attention (B=2, T=2048, D=1024, H=16) on 8 TRN2 NeuronCores.

Sharding: data-parallel over batch (2) x tensor-parallel over head groups (4).
Each core handles 1 batch x 4 heads: Wq/Wk/Wv column-sharded, Wo row-sharded;
each core emits a partial (T, D) output and the host sums 4 partials per batch.

All matmuls run in float32r (1 cycle/row like bf16, ~1.2e-4 relative rounding)
with fp32 PSUM accumulation, and every matmul is padded to the full 128x128
array - the PE clock gate (HAM) only holds 2.4 GHz for high-occupancy streams:

  - S^T = K^T.T @ Q^T uses zero-padded K^T (kTz0 = [k_even; 0], kTz1 =
    [0; k_odd]) so K=128 while the other head's q rows are multiplied by 0.
  - V' is padded to 128 columns per head with parity layouts
      even h: [v(64) | one | 0(63)]  -> A.V rows 0-63 data, row 64 = denom
      odd  h: [one | 0(63) | v(64)]  -> A.V row 0 = denom, rows 64-127 data
    giving M=128 and lane-aligning odd heads to partitions 64-127, so o^T
    lands as [128 f, t] and the Wo projection runs K=128.
  - zero fills come from a DMA'd zeros input (DMA is a legal fp32r producer).

The whole kernel is one fused pipeline over q-slabs: slab ts emits
{x DMA -> PE transposes -> Q/K/V projections}, immediately followed by the
attention steps of qs=ts (which only needs projections of slabs <= ts), so
the scalar engine's exp stream overlaps projection matmuls. Attention steps
are software-pipelined (step i: S matmuls; exp(i-1); A.V(i-2)); denominator
chains (DVE evict -> SBUF-DMA fold -> reciprocal -> unfold -> K=1 rank-1 PE
broadcast -> DVE normalize) and phase-4 output tiles run as deferred closures
spaced through the matmul stream. Preamble DMAs are spread across the
sync/scalar (HWDGE) and gpsimd (SWDGE) queues so x tiles aren't stuck behind
weight/zero transfers.
"""

import sys, os, types

sys.path.insert(0, "/opt/trn_rl_repo")

import numpy as np
from contextlib import ExitStack

import concourse.bass as bass
import concourse.mybir as mybir
import concourse.tile as tile
from concourse import bacc
from concourse.masks import make_identity

B, T, D, H = 2, 2048, 1024, 16
DH = D // H          # 64
NCORES = 8
HG = 4               # heads per core
F = HG * DH          # 256 local features per core
P = 128
F32 = mybir.dt.float32
F32R = mybir.dt.float32r
NEG = -1.0e9

TT = T // P          # 16 t-tiles
QS = T // 512        # 4 q-slabs
DC = D // P          # 8 d-chunks

LAST_RESULTS = None  # BassKernelResults of the most recent hardware run


def _install_ntff_hook():
    if "antenv.axon_hooks" in sys.modules:
        return
    try:
        import antenv
        from trn_agent_boot.trn_boot import _ntff_profile_via_ctypes

        m = types.ModuleType("antenv.axon_hooks")
        h = _ntff_profile_via_ctypes("/opt/axon/libaxon_pjrt.so")
        m.get_axon_ntff_profile_hook = lambda: h
        m.set_axon_ntff_profile_hook = lambda hh: None
        sys.modules["antenv.axon_hooks"] = m
        antenv.axon_hooks = m
    except Exception:
        pass


def build_nc():
    nc = bacc.Bacc("TRN2", target_bir_lowering=False, debug=False)

    x_d = nc.dram_tensor("x", [T, D], F32, kind="ExternalInput").ap()
    wq_d = nc.dram_tensor("wq", [P, DC * F], F32, kind="ExternalInput").ap()
    wk_d = nc.dram_tensor("wk", [P, DC * F], F32, kind="ExternalInput").ap()
    wv_d = nc.dram_tensor("wv", [P, DC * F], F32, kind="ExternalInput").ap()
    wo_d = nc.dram_tensor("wo", [P, 2 * D], F32, kind="ExternalInput").ap()
    bm_d = nc.dram_tensor("bm", [P, 512], F32, kind="ExternalInput").ap()
    y_d = nc.dram_tensor("y", [T, D], F32, kind="ExternalOutput").ap()

    with tile.TileContext(nc) as tc, ExitStack() as ctx:
        const = ctx.enter_context(tc.tile_pool(name="const", bufs=1))
        wpool = ctx.enter_context(tc.tile_pool(name="wpool", bufs=1))
        qkv = ctx.enter_context(tc.tile_pool(name="qkv", bufs=1))
        xpool = ctx.enter_context(tc.tile_pool(name="xpool", bufs=5))
        xsl = ctx.enter_context(tc.tile_pool(name="xsl", bufs=1))
        sp_ps = ctx.enter_context(tc.tile_pool(name="sp_ps", bufs=3, space="PSUM"))
        o_ps = ctx.enter_context(tc.tile_pool(name="o_ps", bufs=2, space="PSUM"))
        ptp = ctx.enter_context(tc.tile_pool(name="ptp", bufs=3))
        stg = ctx.enter_context(tc.tile_pool(name="stg", bufs=3))
        ysb = ctx.enter_context(tc.tile_pool(name="ysb", bufs=2))

        # ---- constants ----
        ident = const.tile([P, P], F32, name="ident")
        make_identity(nc, ident)
        identr = const.tile([P, P], F32R, name="identr")
        nc.scalar.copy(identr[:], ident[:])
        bmask = const.tile([P, 512], F32R, name="bmask")
        ones_f32 = const.tile([P, 64], F32, name="ones_f32")
        nc.vector.memset(ones_f32[:], 1.0)
        osel_f = const.tile([1, 2, P], F32, name="osel_f")
        nc.vector.memset(osel_f[:, 0, 0:64], 1.0)
        nc.vector.memset(osel_f[:, 0, 64:128], 0.0)
        nc.vector.memset(osel_f[:, 1, 0:64], 0.0)
        nc.vector.memset(osel_f[:, 1, 64:128], 1.0)
        osel = const.tile([1, 2, P], F32R, name="osel")
        nc.scalar.copy(osel[:], osel_f[:])
        # touch Exp early so the ACT table load happens in the idle preamble
        warm = const.tile([1, 1], F32, name="warm")
        nc.scalar.activation(warm[:], osel_f[0:1, 0, 0:1], mybir.ActivationFunctionType.Exp)

        # ---- persistent tensors ----
        wq_s = wpool.tile([P, DC, F], F32R, name="wq_s")
        wk_s = wpool.tile([P, DC, F], F32R, name="wk_s")
        wv_s = wpool.tile([P, DC, F], F32R, name="wv_s")
        wo2 = wpool.tile([P, 2, D], F32R, name="wo2")
        qT = qkv.tile([P, 2, T], F32R, name="qT")        # [2 heads x dh, jb, t]
        kTz0 = qkv.tile([P, 2, T], F32R, name="kTz0")    # [k_even; 0]
        kTz1 = qkv.tile([P, 2, T], F32R, name="kTz1")    # [0; k_odd]
        vp = qkv.tile([P, TT, HG, P], F32R, name="vp")   # padded V', parity layouts
        oT = qkv.tile([P, 2, T], F32R, name="oT")        # normalized o^T [f, t]

        # ---- preamble DMAs, spread across queues ----
        # x slab 0 first on sync (gates the first transposes)
        xtiles_next = []
        for tt in range(4):
            xt = xpool.tile([P, D], F32R, name="xt")
            nc.sync.dma_start(out=xt[:], in_=x_d[tt * P:(tt + 1) * P, :].bitcast(F32R))
            xtiles_next.append(xt)
        # weights on scalar-HWDGE; host pre-transposed to partition-major so
        # each is one DMA with 8KB-contiguous per-partition runs
        nc.scalar.dma_start(out=wq_s[:], in_=wq_d[:].bitcast(F32R))
        # zero fills via DVE/ACT copies from a memset F32 staging tile
        # (legal fp32r producers; keeps the DMA rings free for x/weights)
        zeros_f32 = const.tile([P, 512], F32, name="zeros_f32")
        nc.vector.memset(zeros_f32[:], 0.0)
        kz0 = kTz0[64:128, :, :].rearrange("p a b -> p (a b)")
        kz1 = kTz1[0:64, :, :].rearrange("p a b -> p (a b)")
        for c in range(8):
            nc.vector.tensor_copy(kz0[:, c * 512:(c + 1) * 512], zeros_f32[0:64, :])
            nc.scalar.copy(kz1[:, c * 512:(c + 1) * 512], zeros_f32[0:64, :])
        vz = vp[:].rearrange("p a b c -> p (a b c)")
        for c in range(16):
            if c % 2 == 0:
                nc.vector.tensor_copy(vz[:, c * 512:(c + 1) * 512], zeros_f32[:])
            else:
                nc.scalar.copy(vz[:, c * 512:(c + 1) * 512], zeros_f32[:])

        nc.scalar.dma_start(out=bmask[:], in_=bm_d[:].bitcast(F32R))
        # ones columns: even heads col 64, odd heads col 0
        for h in range(HG):
            c = DH if h % 2 == 0 else 0
            nc.scalar.copy(vp[:, :, h, c:c + 1], ones_f32[:, 0:TT])

        # ---- emission helpers ----
        ei = 0

        def emit_slab(ts):
            nonlocal ei, xtiles_next
            xtiles = xtiles_next
            if ts + 1 < QS:
                xtiles_next = []
                for tt in range(4 * (ts + 1), 4 * (ts + 1) + 4):
                    xt = xpool.tile([P, D], F32R, name="xt")
                    nc.sync.dma_start(out=xt[:], in_=x_d[tt * P:(tt + 1) * P, :].bitcast(F32R))
                    xtiles_next.append(xt)
            if ts == 0:
                nc.scalar.dma_start(out=wk_s[:], in_=wk_d[:].bitcast(F32R))
                nc.scalar.dma_start(out=wv_s[:], in_=wv_d[:].bitcast(F32R))
                nc.scalar.dma_start(out=wo2[:], in_=wo_d[:].bitcast(F32R))
            sl = slice(ts * 512, (ts + 1) * 512)
            xTs = xsl.tile([P, DC, 512], F32R, name="xTs")
            for dc in range(DC):
                tp = sp_ps.tile([P, 512], F32R, name="tp", tag="sp")
                for i, xt in enumerate(xtiles):
                    nc.tensor.transpose(tp[:, i * P:(i + 1) * P], xt[:, dc * P:(dc + 1) * P], identr[:])
                if ei % 2 == 0:
                    nc.vector.tensor_copy(xTs[:, dc, :], tp[:])
                else:
                    nc.scalar.copy(xTs[:, dc, :], tp[:])
                ei += 1
            for w_s, which in ((wq_s, "q"), (wk_s, "k")):
                for jb in range(2):
                    pp = sp_ps.tile([P, 512], F32, name="pp", tag="sp")
                    for dc in range(DC):
                        nc.tensor.matmul(
                            out=pp[:],
                            lhsT=w_s[:, dc, jb * P:(jb + 1) * P],
                            rhs=xTs[:, dc, :],
                            start=(dc == 0),
                            stop=(dc == DC - 1),
                        )
                    if which == "q":
                        if ei % 2 == 0:
                            nc.vector.tensor_copy(qT[:, jb, sl], pp[:])
                        else:
                            nc.scalar.copy(qT[:, jb, sl], pp[:])
                    else:
                        if ei % 2 == 0:
                            nc.vector.tensor_copy(kTz0[0:64, jb, sl], pp[0:64, :])
                            nc.scalar.copy(kTz1[64:128, jb, sl], pp[64:128, :])
                        else:
                            nc.scalar.copy(kTz0[0:64, jb, sl], pp[0:64, :])
                            nc.vector.tensor_copy(kTz1[64:128, jb, sl], pp[64:128, :])
                    ei += 1
            for j, tt in enumerate(range(4 * ts, 4 * ts + 4)):
                pv = sp_ps.tile([P, F], F32, name="pv", tag="sp")
                for dc in range(DC):
                    nc.tensor.matmul(
                        out=pv[:],
                        lhsT=xTs[:, dc, j * P:(j + 1) * P],
                        rhs=wv_s[:, dc, :],
                        start=(dc == 0),
                        stop=(dc == DC - 1),
                    )
                pvv = pv[:].rearrange("p (hp par dh) -> p hp par dh", hp=2, par=2, dh=DH)
                ve = vp[:, tt, :, :].rearrange("p (hp par) c -> p hp par c", par=2)
                if ei % 2 == 0:
                    nc.vector.tensor_copy(ve[:, :, 0, 0:DH], pvv[:, :, 0, :])
                    nc.scalar.copy(ve[:, :, 1, DH:P], pvv[:, :, 1, :])
                else:
                    nc.scalar.copy(ve[:, :, 0, 0:DH], pvv[:, :, 0, :])
                    nc.vector.tensor_copy(ve[:, :, 1, DH:P], pvv[:, :, 1, :])
                ei += 1

        state = {}

        def emit_S(qs, h, kp):
            jbh, par = h // 2, h % 2
            kTz = kTz0 if par == 0 else kTz1
            q0 = qs * 512
            spair = sp_ps.tile([P, 1024], F32, name="spair", tag="sp")
            for half in range(2):
                kt = 2 * kp + half
                k0 = kt * P
                sreg = spair[:, half * 512:(half + 1) * 512]
                lhsk = kTz[:, jbh, k0:k0 + P]
                rhsq = qT[:, jbh, :]
                if k0 >= q0:
                    d = k0 - q0
                    nc.tensor.matmul(out=sreg[:, 0:d + P], lhsT=identr[:],
                                     rhs=bmask[:, 384 - d:512], start=True, stop=False)
                    nc.tensor.matmul(out=sreg[:, d:d + P], lhsT=lhsk,
                                     rhs=rhsq[:, q0 + d:q0 + d + P],
                                     start=False, stop=(d == 384))
                    if d < 384:
                        nc.tensor.matmul(out=sreg[:, d + P:512], lhsT=lhsk,
                                         rhs=rhsq[:, q0 + d + P:q0 + 512],
                                         start=False, stop=True)
                else:
                    nc.tensor.matmul(out=sreg, lhsT=lhsk,
                                     rhs=rhsq[:, q0:q0 + 512],
                                     start=True, stop=True)
            state[(qs, h, kp)] = spair

        def emit_exp(qs, h, kp):
            spair = state[(qs, h, kp)]
            pt = ptp.tile([P, 1024], F32R, name="pt")
            nc.scalar.activation(pt[:], spair[:], mybir.ActivationFunctionType.Exp, scale=0.125)
            state[(qs, h, kp)] = (spair, pt)

        def emit_AV(qs, h, kp, nkt):
            _, pt = state.pop((qs, h, kp))
            q0 = qs * 512
            if kp == 0:
                state[(qs, h)] = o_ps.tile([P, 512], F32, name="opsum")
            opsum = state[(qs, h)]
            for half in range(2):
                kt = 2 * kp + half
                d = max(kt * P - q0, 0)
                nc.tensor.matmul(
                    out=opsum[:, d:512],
                    lhsT=vp[:, kt, h, :],
                    rhs=pt[:, half * 512 + d:half * 512 + 512],
                    start=(kt == 0),
                    stop=(kt == nkt - 1),
                )

        def emit_normA(qs, h):
            opsum = state[(qs, h)]
            r = DH if h % 2 == 0 else 0
            dstage = stg.tile([DH + 1, 512], F32, name="dstage")
            nc.vector.tensor_copy(dstage[r:r + 1, :], opsum[r:r + 1, :])
            dfold = stg.tile([8, 64], F32, name="dfold")
            nc.gpsimd.dma_start(out=dfold[:], in_=dstage[r:r + 1, :])
            nc.vector.reciprocal(dfold[:], dfold[:])
            inv = stg.tile([1, 512], F32R, name="inv")
            nc.gpsimd.dma_start(out=inv[:], in_=dfold[:].bitcast(F32R))
            state[(qs, h, "inv")] = inv

        def emit_normB(qs, h):
            opsum = state.pop((qs, h))
            inv = state.pop((qs, h, "inv"))
            jbh, par = h // 2, h % 2
            q0 = qs * 512
            rows = slice(0, DH) if par == 0 else slice(DH, P)
            bcast = sp_ps.tile([P, 512], F32, name="bcast", tag="sp")
            nc.tensor.matmul(out=bcast[:], lhsT=osel[:, par, :], rhs=inv[:], start=True, stop=True)
            bsb = stg.tile([P, 512], F32, name="bsb")
            nc.vector.tensor_copy(bsb[rows, :], bcast[rows, :])
            nc.vector.tensor_mul(oT[rows, jbh, q0:q0 + 512], opsum[rows, :], bsb[rows, :])

        def emit_ytile(qs, tt, e):
            yp = sp_ps.tile([P, 1024], F32, name="yp", tag="sp")
            for jh in range(2):
                for fc in range(2):
                    nc.tensor.matmul(
                        out=yp[:, jh * 512:(jh + 1) * 512],
                        lhsT=oT[:, fc, tt * P:(tt + 1) * P],
                        rhs=wo2[:, fc, jh * 512:(jh + 1) * 512],
                        start=(fc == 0),
                        stop=(fc == 1),
                    )
            for jh in range(2):
                yt = ysb.tile([P, 512], F32, name="yt")
                if (e + jh) % 2 == 0:
                    nc.vector.tensor_copy(yt[:], yp[:, jh * 512:(jh + 1) * 512])
                else:
                    nc.scalar.copy(yt[:], yp[:, jh * 512:(jh + 1) * 512])
                nc.scalar.dma_start(out=y_d[tt * P:(tt + 1) * P, jh * 512:(jh + 1) * 512], in_=yt[:])

        # ---- fused pipeline ----
        steps = []
        for qs in range(QS):
            for h in range(HG):
                nkt = 4 * qs + 4
                for kp in range(nkt // 2):
                    steps.append((qs, h, kp, nkt))
        first_step_of_qs = {}
        for i, (qs, h, kp, nkt) in enumerate(steps):
            if (h, kp) == (0, 0):
                first_step_of_qs[i] = qs

        todo = []

        def flush(i):
            while todo and todo[0][0] <= i:
                todo.pop(0)[1]()

        nsteps = len(steps)
        for i in range(nsteps):
            if i in first_step_of_qs:
                emit_slab(first_step_of_qs[i])
            qs, h, kp, nkt = steps[i]
            emit_S(qs, h, kp)
            flush(i)
            if i >= 1:
                pqs, ph_, pkp, _ = steps[i - 1]
                emit_exp(pqs, ph_, pkp)
            if i >= 2:
                pqs, ph_, pkp, pnkt = steps[i - 2]
                emit_AV(pqs, ph_, pkp, pnkt)
                if pkp == pnkt // 2 - 1:
                    emit_normA(pqs, ph_)
                    todo.append((i + min(5, 2 * (pqs + 1) + 1), lambda q=pqs, hh=ph_: emit_normB(q, hh)))
                    if ph_ == HG - 1:
                        for j, tt in enumerate(range(4 * pqs, 4 * pqs + 4)):
                            todo.append((i + 6 + j,
                                         lambda q=pqs, t_=tt, e=j: emit_ytile(q, t_, e)))
        # drain
        emit_exp(*steps[nsteps - 1][:3])
        for i in (nsteps - 2, nsteps - 1):
            qs, h, kp, nkt = steps[i]
            emit_AV(qs, h, kp, nkt)
            if kp == nkt // 2 - 1:
                emit_normA(qs, h)
                todo.append((10 ** 9, lambda q=qs, hh=h: emit_normB(q, hh)))
                if h == HG - 1:
                    for j, tt in enumerate(range(4 * qs, 4 * qs + 4)):
                        todo.append((10 ** 9, lambda q=qs, t_=tt, e=j: emit_ytile(q, t_, e)))
        for _, fn in todo:
            fn()

    nc.compile()
    return nc


def make_mask():
    # BM[k, j] = -1e9 if j < 384 + k else 0
    j = np.arange(512)[None, :]
    k = np.arange(P)[:, None]
    return np.where(j < 384 + k, np.float32(NEG), np.float32(0.0)).astype(np.float32)


def make_core_inputs(x, Wq, Wk, Wv, Wo):
    bm = make_mask()
    in_maps = []
    for c in range(NCORES):
        b, hg = c // HG, c % HG
        s = slice(hg * F, (hg + 1) * F)

        def pmajor(w, chunks):  # [chunks*P, f] -> [P, chunks*f]
            return np.ascontiguousarray(
                w.reshape(chunks, P, -1).transpose(1, 0, 2).reshape(P, -1))

        in_maps.append({
            "x": np.ascontiguousarray(x[b]),
            "wq": pmajor(Wq[:, s], DC),
            "wk": pmajor(Wk[:, s], DC),
            "wv": pmajor(Wv[:, s], DC),
            "wo": pmajor(Wo[s, :], 2),
            "bm": bm,
        })
    return in_maps


_NC_CACHE = None


def _get_nc():
    global _NC_CACHE
    if _NC_CACHE is None:
        _NC_CACHE = build_nc()
    return _NC_CACHE


def kernel(x, Wq, Wk, Wv, Wo):
    global LAST_RESULTS
    _install_ntff_hook()
    from concourse.bass_utils import run_bass_kernel_spmd

    x = np.asarray(x, dtype=np.float32)
    Wq = np.asarray(Wq, dtype=np.float32)
    Wk = np.asarray(Wk, dtype=np.float32)
    Wv = np.asarray(Wv, dtype=np.float32)
    Wo = np.asarray(Wo, dtype=np.float32)

    nc = _get_nc()
    in_maps = make_core_inputs(x, Wq, Wk, Wv, Wo)
    res = run_bass_kernel_spmd(nc, in_maps, list(range(NCORES)))
    LAST_RESULTS = res

    out = np.zeros((B, T, D), dtype=np.float32)
    for c in range(NCORES):
        out[c // HG] += res.results[c]["y"]
    return out


# revision 30
# speedup vs baseline: 1.0233x; 1.0233x over previous
"""Causal self-attention (B=2, T=2048, D=1024, H=16) on 8 TRN2 NeuronCores.

Sharding: data-parallel over batch (2) x tensor-parallel over head groups (4).
Each core handles 1 batch x 4 heads: Wq/Wk/Wv column-sharded, Wo row-sharded;
each core emits a partial (T, D) output and the host sums 4 partials per batch.

All matmuls run in float32r (1 cycle/row like bf16, ~1.2e-4 relative rounding)
with fp32 PSUM accumulation, and every matmul is padded to the full 128x128
array - the PE clock gate (HAM) only holds 2.4 GHz for high-occupancy streams:

  - S^T = K^T.T @ Q^T uses zero-padded K^T (kTz0 = [k_even; 0], kTz1 =
    [0; k_odd]) so K=128 while the other head's q rows are multiplied by 0.
  - V' is padded to 128 columns per head with parity layouts
      even h: [v(64) | one | 0(63)]  -> A.V rows 0-63 data, row 64 = denom
      odd  h: [one | 0(63) | v(64)]  -> A.V row 0 = denom, rows 64-127 data
    giving M=128 and lane-aligning odd heads to partitions 64-127, so o^T
    lands as [128 f, t] and the Wo projection runs K=128.
  - zero fills come from a DMA'd zeros input (DMA is a legal fp32r producer).

The whole kernel is one fused pipeline over q-slabs: slab ts emits
{x DMA -> PE transposes -> Q/K/V projections}, immediately followed by the
attention steps of qs=ts (which only needs projections of slabs <= ts), so
the scalar engine's exp stream overlaps projection matmuls. Attention steps
are software-pipelined (step i: S matmuls; exp(i-1); A.V(i-2)); denominator
chains (DVE evict -> SBUF-DMA fold -> reciprocal -> unfold -> K=1 rank-1 PE
broadcast -> DVE normalize) and phase-4 output tiles run as deferred closures
spaced through the matmul stream. Preamble DMAs are spread across the
sync/scalar (HWDGE) and gpsimd (SWDGE) queues so x tiles aren't stuck behind
weight/zero transfers.
"""

import sys, os, types

sys.path.insert(0, "/opt/trn_rl_repo")

import numpy as np
from contextlib import ExitStack

import concourse.bass as bass
import concourse.mybir as mybir
import concourse.tile as tile
from concourse import bacc
from concourse.masks import make_identity

B, T, D, H = 2, 2048, 1024, 16
DH = D // H          # 64
NCORES = 8
HG = 4               # heads per core
F = HG * DH          # 256 local features per core
P = 128
F32 = mybir.dt.float32
F32R = mybir.dt.float32r
NEG = -1.0e9

TT = T // P          # 16 t-tiles
QS = T // 512        # 4 q-slabs
DC = D // P          # 8 d-chunks

LAST_RESULTS = None  # BassKernelResults of the most recent hardware run


def _install_ntff_hook():
    if "antenv.axon_hooks" in sys.modules:
        return
    try:
        import antenv
        from trn_agent_boot.trn_boot import _ntff_profile_via_ctypes

        m = types.ModuleType("antenv.axon_hooks")
        h = _ntff_profile_via_ctypes("/opt/axon/libaxon_pjrt.so")
        m.get_axon_ntff_profile_hook = lambda: h
        m.set_axon_ntff_profile_hook = lambda hh: None
        sys.modules["antenv.axon_hooks"] = m
        antenv.axon_hooks = m
    except Exception:
        pass


def build_nc():
    nc = bacc.Bacc("TRN2", target_bir_lowering=False, debug=False)

    x_d = nc.dram_tensor("x", [T, D], F32, kind="ExternalInput").ap()
    wq_d = nc.dram_tensor("wq", [P, DC * F], F32, kind="ExternalInput").ap()
    wk_d = nc.dram_tensor("wk", [P, DC * F], F32, kind="ExternalInput").ap()
    wv_d = nc.dram_tensor("wv", [P, DC * F], F32, kind="ExternalInput").ap()
    wo_d = nc.dram_tensor("wo", [P, 2 * D], F32, kind="ExternalInput").ap()
    bm_d = nc.dram_tensor("bm", [P, 512], F32, kind="ExternalInput").ap()
    y_d = nc.dram_tensor("y", [T, D], F32, kind="ExternalOutput").ap()

    with tile.TileContext(nc) as tc, ExitStack() as ctx:
        const = ctx.enter_context(tc.tile_pool(name="const", bufs=1))
        wpool = ctx.enter_context(tc.tile_pool(name="wpool", bufs=1))
        qkv = ctx.enter_context(tc.tile_pool(name="qkv", bufs=1))
        xpool = ctx.enter_context(tc.tile_pool(name="xpool", bufs=4))
        xsl = ctx.enter_context(tc.tile_pool(name="xsl", bufs=1))
        sp_ps = ctx.enter_context(tc.tile_pool(name="sp_ps", bufs=3, space="PSUM"))
        o_ps = ctx.enter_context(tc.tile_pool(name="o_ps", bufs=2, space="PSUM"))
        ptp = ctx.enter_context(tc.tile_pool(name="ptp", bufs=3))
        stg = ctx.enter_context(tc.tile_pool(name="stg", bufs=3))
        ysb = ctx.enter_context(tc.tile_pool(name="ysb", bufs=2))

        # ---- constants ----
        ident = const.tile([P, P], F32, name="ident")
        make_identity(nc, ident)
        identr = const.tile([P, P], F32R, name="identr")
        nc.scalar.copy(identr[:], ident[:])
        bmask = const.tile([P, 512], F32R, name="bmask")
        ones_f32 = const.tile([P, 64], F32, name="ones_f32")
        nc.vector.memset(ones_f32[:], 1.0)
        osel_f = const.tile([1, 2, P], F32, name="osel_f")
        nc.vector.memset(osel_f[:, 0, 0:64], 1.0)
        nc.vector.memset(osel_f[:, 0, 64:128], 0.0)
        nc.vector.memset(osel_f[:, 1, 0:64], 0.0)
        nc.vector.memset(osel_f[:, 1, 64:128], 1.0)
        osel = const.tile([1, 2, P], F32R, name="osel")
        nc.scalar.copy(osel[:], osel_f[:])
        # touch Exp early so the ACT table load happens in the idle preamble
        warm = const.tile([1, 1], F32, name="warm")
        nc.scalar.activation(warm[:], osel_f[0:1, 0, 0:1], mybir.ActivationFunctionType.Exp)

        # ---- persistent tensors ----
        wq_s = wpool.tile([P, DC, F], F32R, name="wq_s")
        wk_s = wpool.tile([P, DC, F], F32R, name="wk_s")
        wv_s = wpool.tile([P, DC, F], F32R, name="wv_s")
        wo2 = wpool.tile([P, 2, D], F32R, name="wo2")
        qT = qkv.tile([P, 2, T], F32R, name="qT")        # [2 heads x dh, jb, t]
        kTz0 = qkv.tile([P, 2, T], F32R, name="kTz0")    # [k_even; 0]
        kTz1 = qkv.tile([P, 2, T], F32R, name="kTz1")    # [0; k_odd]
        vp = qkv.tile([P, TT, HG, P], F32R, name="vp")   # padded V', parity layouts
        oT = qkv.tile([P, 2, T], F32R, name="oT")        # normalized o^T [f, t]

        # ---- preamble DMAs, spread across queues ----
        # x slab 0 first on sync (gates the first transposes)
        xtiles_next = []
        for tt in range(4):
            xt = xpool.tile([P, D], F32R, name="xt")
            nc.sync.dma_start(out=xt[:], in_=x_d[tt * P:(tt + 1) * P, :].bitcast(F32R))
            xtiles_next.append(xt)
        # weights on scalar-HWDGE; host pre-transposed to partition-major so
        # each is one DMA with 8KB-contiguous per-partition runs
        nc.scalar.dma_start(out=wq_s[:], in_=wq_d[:].bitcast(F32R))
        nc.scalar.dma_start(out=wk_s[:], in_=wk_d[:].bitcast(F32R))
        # zero fills via DVE/ACT copies from a memset F32 staging tile
        # (legal fp32r producers; keeps the DMA rings free for x/weights)
        zeros_f32 = const.tile([P, 512], F32, name="zeros_f32")
        nc.vector.memset(zeros_f32[:], 0.0)
        kz0 = kTz0[64:128, :, :].rearrange("p a b -> p (a b)")
        kz1 = kTz1[0:64, :, :].rearrange("p a b -> p (a b)")
        for c in range(8):
            nc.vector.tensor_copy(kz0[:, c * 512:(c + 1) * 512], zeros_f32[0:64, :])
            nc.scalar.copy(kz1[:, c * 512:(c + 1) * 512], zeros_f32[0:64, :])
        vz = vp[:].rearrange("p a b c -> p (a b c)")
        for c in range(16):
            if c % 2 == 0:
                nc.vector.tensor_copy(vz[:, c * 512:(c + 1) * 512], zeros_f32[:])
            else:
                nc.scalar.copy(vz[:, c * 512:(c + 1) * 512], zeros_f32[:])

        nc.scalar.dma_start(out=bmask[:], in_=bm_d[:].bitcast(F32R))
        # ones columns: even heads col 64, odd heads col 0
        for h in range(HG):
            c = DH if h % 2 == 0 else 0
            nc.scalar.copy(vp[:, :, h, c:c + 1], ones_f32[:, 0:TT])

        # ---- emission helpers ----
        ei = 0

        def emit_slab(ts):
            nonlocal ei, xtiles_next
            xtiles = xtiles_next
            if ts + 1 < QS:
                xtiles_next = []
                for tt in range(4 * (ts + 1), 4 * (ts + 1) + 4):
                    xt = xpool.tile([P, D], F32R, name="xt")
                    nc.sync.dma_start(out=xt[:], in_=x_d[tt * P:(tt + 1) * P, :].bitcast(F32R))
                    xtiles_next.append(xt)
            if ts == 0:
                nc.scalar.dma_start(out=wv_s[:], in_=wv_d[:].bitcast(F32R))
                nc.scalar.dma_start(out=wo2[:], in_=wo_d[:].bitcast(F32R))
            sl = slice(ts * 512, (ts + 1) * 512)
            xTs = xsl.tile([P, DC, 512], F32R, name="xTs")
            for dc in range(DC):
                tp = sp_ps.tile([P, 512], F32R, name="tp", tag="sp")
                for i, xt in enumerate(xtiles):
                    nc.tensor.transpose(tp[:, i * P:(i + 1) * P], xt[:, dc * P:(dc + 1) * P], identr[:])
                if ei % 2 == 0:
                    nc.vector.tensor_copy(xTs[:, dc, :], tp[:])
                else:
                    nc.scalar.copy(xTs[:, dc, :], tp[:])
                ei += 1
            for w_s, which in ((wq_s, "q"), (wk_s, "k")):
                for jb in range(2):
                    pp = sp_ps.tile([P, 512], F32, name="pp", tag="sp")
                    for dc in range(DC):
                        nc.tensor.matmul(
                            out=pp[:],
                            lhsT=w_s[:, dc, jb * P:(jb + 1) * P],
                            rhs=xTs[:, dc, :],
                            start=(dc == 0),
                            stop=(dc == DC - 1),
                        )
                    if which == "q":
                        if ei % 2 == 0:
                            nc.vector.tensor_copy(qT[:, jb, sl], pp[:])
                        else:
                            nc.scalar.copy(qT[:, jb, sl], pp[:])
                    else:
                        if ei % 2 == 0:
                            nc.vector.tensor_copy(kTz0[0:64, jb, sl], pp[0:64, :])
                            nc.scalar.copy(kTz1[64:128, jb, sl], pp[64:128, :])
                        else:
                            nc.scalar.copy(kTz0[0:64, jb, sl], pp[0:64, :])
                            nc.vector.tensor_copy(kTz1[64:128, jb, sl], pp[64:128, :])
                    ei += 1
            for j, tt in enumerate(range(4 * ts, 4 * ts + 4)):
                pv = sp_ps.tile([P, F], F32, name="pv", tag="sp")
                for dc in range(DC):
                    nc.tensor.matmul(
                        out=pv[:],
                        lhsT=xTs[:, dc, j * P:(j + 1) * P],
                        rhs=wv_s[:, dc, :],
                        start=(dc == 0),
                        stop=(dc == DC - 1),
                    )
                pvv = pv[:].rearrange("p (hp par dh) -> p hp par dh", hp=2, par=2, dh=DH)
                ve = vp[:, tt, :, :].rearrange("p (hp par) c -> p hp par c", par=2)
                if ei % 2 == 0:
                    nc.vector.tensor_copy(ve[:, :, 0, 0:DH], pvv[:, :, 0, :])
                    nc.scalar.copy(ve[:, :, 1, DH:P], pvv[:, :, 1, :])
                else:
                    nc.scalar.copy(ve[:, :, 0, 0:DH], pvv[:, :, 0, :])
                    nc.vector.tensor_copy(ve[:, :, 1, DH:P], pvv[:, :, 1, :])
                ei += 1

        state = {}

        def emit_S(qs, h, kp):
            jbh, par = h // 2, h % 2
            kTz = kTz0 if par == 0 else kTz1
            q0 = qs * 512
            spair = sp_ps.tile([P, 1024], F32, name="spair", tag="sp")
            for half in range(2):
                kt = 2 * kp + half
                k0 = kt * P
                sreg = spair[:, half * 512:(half + 1) * 512]
                lhsk = kTz[:, jbh, k0:k0 + P]
                rhsq = qT[:, jbh, :]
                if k0 >= q0:
                    d = k0 - q0
                    nc.tensor.matmul(out=sreg[:, 0:d + P], lhsT=identr[:],
                                     rhs=bmask[:, 384 - d:512], start=True, stop=False)
                    nc.tensor.matmul(out=sreg[:, d:d + P], lhsT=lhsk,
                                     rhs=rhsq[:, q0 + d:q0 + d + P],
                                     start=False, stop=(d == 384))
                    if d < 384:
                        nc.tensor.matmul(out=sreg[:, d + P:512], lhsT=lhsk,
                                         rhs=rhsq[:, q0 + d + P:q0 + 512],
                                         start=False, stop=True)
                else:
                    nc.tensor.matmul(out=sreg, lhsT=lhsk,
                                     rhs=rhsq[:, q0:q0 + 512],
                                     start=True, stop=True)
            state[(qs, h, kp)] = spair

        def emit_exp(qs, h, kp):
            spair = state[(qs, h, kp)]
            pt = ptp.tile([P, 1024], F32R, name="pt")
            nc.scalar.activation(pt[:], spair[:], mybir.ActivationFunctionType.Exp, scale=0.125)
            state[(qs, h, kp)] = (spair, pt)

        def emit_AV(qs, h, kp, nkt):
            _, pt = state.pop((qs, h, kp))
            q0 = qs * 512
            if kp == 0:
                state[(qs, h)] = o_ps.tile([P, 512], F32, name="opsum")
            opsum = state[(qs, h)]
            for half in range(2):
                kt = 2 * kp + half
                d = max(kt * P - q0, 0)
                nc.tensor.matmul(
                    out=opsum[:, d:512],
                    lhsT=vp[:, kt, h, :],
                    rhs=pt[:, half * 512 + d:half * 512 + 512],
                    start=(kt == 0),
                    stop=(kt == nkt - 1),
                )

        def emit_normA(qs, h):
            opsum = state[(qs, h)]
            r = DH if h % 2 == 0 else 0
            dstage = stg.tile([DH + 1, 512], F32, name="dstage")
            nc.vector.tensor_copy(dstage[r:r + 1, :], opsum[r:r + 1, :])
            dfold = stg.tile([8, 64], F32, name="dfold")
            nc.gpsimd.dma_start(out=dfold[:], in_=dstage[r:r + 1, :])
            nc.vector.reciprocal(dfold[:], dfold[:])
            inv = stg.tile([1, 512], F32R, name="inv")
            nc.gpsimd.dma_start(out=inv[:], in_=dfold[:].bitcast(F32R))
            state[(qs, h, "inv")] = inv

        def emit_normB(qs, h):
            opsum = state.pop((qs, h))
            inv = state.pop((qs, h, "inv"))
            jbh, par = h // 2, h % 2
            q0 = qs * 512
            rows = slice(0, DH) if par == 0 else slice(DH, P)
            bcast = sp_ps.tile([P, 512], F32, name="bcast", tag="sp")
            nc.tensor.matmul(out=bcast[:], lhsT=osel[:, par, :], rhs=inv[:], start=True, stop=True)
            bsb = stg.tile([P, 512], F32, name="bsb")
            nc.vector.tensor_copy(bsb[rows, :], bcast[rows, :])
            nc.vector.tensor_mul(oT[rows, jbh, q0:q0 + 512], opsum[rows, :], bsb[rows, :])

        def emit_ytile(qs, tt, e):
            yp = sp_ps.tile([P, 1024], F32, name="yp", tag="sp")
            for jh in range(2):
                for fc in range(2):
                    nc.tensor.matmul(
                        out=yp[:, jh * 512:(jh + 1) * 512],
                        lhsT=oT[:, fc, tt * P:(tt + 1) * P],
                        rhs=wo2[:, fc, jh * 512:(jh + 1) * 512],
                        start=(fc == 0),
                        stop=(fc == 1),
                    )
            yt = ysb.tile([P, D], F32, name="yt")
            if e % 2 == 0:
                nc.vector.tensor_copy(yt[:], yp[:])
            else:
                nc.scalar.copy(yt[:], yp[:])
            nc.scalar.dma_start(out=y_d[tt * P:(tt + 1) * P, :], in_=yt[:])

        # ---- fused pipeline ----
        steps = []
        for qs in range(QS):
            for h in range(HG):
                nkt = 4 * qs + 4
                for kp in range(nkt // 2):
                    steps.append((qs, h, kp, nkt))
        first_step_of_qs = {}
        for i, (qs, h, kp, nkt) in enumerate(steps):
            if (h, kp) == (0, 0):
                first_step_of_qs[i] = qs

        todo = []

        def flush(i):
            while todo and todo[0][0] <= i:
                todo.pop(0)[1]()

        nsteps = len(steps)
        for i in range(nsteps):
            if i in first_step_of_qs:
                emit_slab(first_step_of_qs[i])
            qs, h, kp, nkt = steps[i]
            emit_S(qs, h, kp)
            flush(i)
            if i >= 1:
                pqs, ph_, pkp, _ = steps[i - 1]
                emit_exp(pqs, ph_, pkp)
            if i >= 2:
                pqs, ph_, pkp, pnkt = steps[i - 2]
                emit_AV(pqs, ph_, pkp, pnkt)
                if pkp == pnkt // 2 - 1:
                    emit_normA(pqs, ph_)
                    todo.append((i + min(5, 2 * (pqs + 1) + 1), lambda q=pqs, hh=ph_: emit_normB(q, hh)))
                    if ph_ == HG - 1:
                        for j, tt in enumerate(range(4 * pqs, 4 * pqs + 4)):
                            todo.append((i + 6 + j,
                                         lambda q=pqs, t_=tt, e=j: emit_ytile(q, t_, e)))
        # drain
        emit_exp(*steps[nsteps - 1][:3])
        for i in (nsteps - 2, nsteps - 1):
            qs, h, kp, nkt = steps[i]
            emit_AV(qs, h, kp, nkt)
            if kp == nkt // 2 - 1:
                emit_normA(qs, h)
                todo.append((10 ** 9, lambda q=qs, hh=h: emit_normB(q, hh)))
                if h == HG - 1:
                    for j, tt in enumerate(range(4 * qs, 4 * qs + 4)):
                        todo.append((10 ** 9, lambda q=qs, t_=tt, e=j: emit_ytile(q, t_, e)))
        for _, fn in todo:
            fn()

    nc.compile()
    return nc


def make_mask():
    # BM[k, j] = -1e9 if j < 384 + k else 0
    j = np.arange(512)[None, :]
    k = np.arange(P)[:, None]
    return np.where(j < 384 + k, np.float32(NEG), np.float32(0.0)).astype(np.float32)


def make_core_inputs(x, Wq, Wk, Wv, Wo):
    bm = make_mask()
    in_maps = []
    for c in range(NCORES):
        b, hg = c // HG, c % HG
        s = slice(hg * F, (hg + 1) * F)

        def pmajor(w, chunks):  # [chunks*P, f] -> [P, chunks*f]
            return np.ascontiguousarray(
                w.reshape(chunks, P, -1).transpose(1, 0, 2).reshape(P, -1))

        in_maps.append({
            "x": np.ascontiguousarray(x[b]),
            "wq": pmajor(Wq[:, s], DC),
            "wk": pmajor(Wk[:, s], DC),
            "wv": pmajor(Wv[:, s], DC),
            "wo": pmajor(Wo[s, :], 2),
            "bm": bm,
        })
    return in_maps


_NC_CACHE = None


def _get_nc():
    global _NC_CACHE
    if _NC_CACHE is None:
        _NC_CACHE = build_nc()
    return _NC_CACHE


def kernel(x, Wq, Wk, Wv, Wo):
    global LAST_RESULTS
    _install_ntff_hook()
    from concourse.bass_utils import run_bass_kernel_spmd

    x = np.asarray(x, dtype=np.float32)
    Wq = np.asarray(Wq, dtype=np.float32)
    Wk = np.asarray(Wk, dtype=np.float32)
    Wv = np.asarray(Wv, dtype=np.float32)
    Wo = np.asarray(Wo, dtype=np.float32)

    nc = _get_nc()
    in_maps = make_core_inputs(x, Wq, Wk, Wv, Wo)
    res = run_bass_kernel_spmd(nc, in_maps, list(range(NCORES)))
    LAST_RESULTS = res

    out = np.zeros((B, T, D), dtype=np.float32)
    for c in range(NCORES):
        out[c // HG] += res.results[c]["y"]
    return out
